# revision 1
# baseline (speedup 1.0000x reference)
"""CLIP text embedding lookup on 8 TRN2 NeuronCores.

out[1, 77, 768] = token_weight[input_ids] + position_weight[position_ids]

Strategy: sequence-parallel. 77 positions are padded to 80 and split 10 per
core. Each core indirect-DMA-gathers its 10 token rows from the full
replicated [49408, 768] table (one descriptor per row, one dest partition per
row), adds the core's 10 position-embedding rows (pre-sharded on the host —
position_ids is a static arange; a general fallback gathers them on the host
only if a caller passes a permuted position_ids), and writes its [10, 768]
output slice. The host concatenates the 8 slices and trims to 77 rows.
"""

import numpy as np

NCORES = 8
SEQ = 77
DIM = 768
VOCAB = 49408
MAX_POS = 77
ROWS = 10  # ceil(77 / 8)
PAD_SEQ = NCORES * ROWS  # 80

# test.py can flip TRACE; LAST_RESULTS stashes BassKernelResults for test.py.
TRACE = False
LAST_RESULTS = None

_compiled = None


def _build():
    import concourse.bacc as bacc
    import concourse.bass as bass
    import concourse.mybir as mybir
    import concourse.tile as tile

    nc = bacc.Bacc(
        "TRN2", target_bir_lowering=False, debug=False, num_devices=NCORES
    )
    idx = nc.dram_tensor("idx", [ROWS, 1], mybir.dt.int32, kind="ExternalInput").ap()
    table = nc.dram_tensor(
        "table", [VOCAB, DIM], mybir.dt.float32, kind="ExternalInput"
    ).ap()
    pos = nc.dram_tensor(
        "pos", [ROWS, DIM], mybir.dt.float32, kind="ExternalInput"
    ).ap()
    out = nc.dram_tensor(
        "out", [ROWS, DIM], mybir.dt.float32, kind="ExternalOutput"
    ).ap()

    with tile.TileContext(nc) as tc:
        with tc.tile_pool(name="sbuf", bufs=1) as pool:
            idx_t = pool.tile([ROWS, 1], mybir.dt.int32)
            tok_t = pool.tile([ROWS, DIM], mybir.dt.float32)
            # idx load rides the gather's own SWDGE queue (no cross-engine
            # hop before the gather). Position rows go straight to the DRAM
            # output on the SP HWDGE queue, overlapping the gather chain;
            # the gathered token rows are then accumulated into the output
            # by the SWDGE CCE (accum_op=add), which removes both the
            # vector-add and its cross-engine semaphore hop.
            nc.gpsimd.dma_start(out=idx_t[:], in_=idx[:])
            nc.sync.dma_start(out=out[:], in_=pos[:])
            nc.gpsimd.indirect_dma_start(
                out=tok_t[:],
                out_offset=None,
                in_=table[:],
                in_offset=bass.IndirectOffsetOnAxis(ap=idx_t[:, :1], axis=0),
            )
            nc.gpsimd.dma_start(
                out=out[:], in_=tok_t[:], accum_op=mybir.AluOpType.add
            )
    nc.compile()
    return nc


def kernel(**inputs) -> np.ndarray:
    global _compiled, LAST_RESULTS
    from concourse.bass_utils import run_bass_kernel_spmd

    input_ids = np.asarray(inputs["input_ids"]).astype(np.int32).reshape(-1)
    position_ids = np.asarray(inputs["position_ids"]).astype(np.int64).reshape(-1)
    token_weight = np.ascontiguousarray(
        np.asarray(inputs["token_weight"], dtype=np.float32)
    )
    position_weight = np.ascontiguousarray(
        np.asarray(inputs["position_weight"], dtype=np.float32)
    )

    if _compiled is None:
        _compiled = _build()
    nc = _compiled

    ids_pad = np.zeros(PAD_SEQ, np.int32)
    ids_pad[:SEQ] = input_ids
    # Shard the (replicated) position table by sequence position. For the
    # canonical arange position_ids this is a pure row-shard; any other
    # permutation is resolved host-side the same way.
    pos_rows = position_weight[position_ids]  # [SEQ, DIM]
    pos_pad = np.zeros((PAD_SEQ, DIM), np.float32)
    pos_pad[:SEQ] = pos_rows

    in_maps = []
    for c in range(NCORES):
        sl = slice(c * ROWS, (c + 1) * ROWS)
        in_maps.append(
            {
                "idx": ids_pad[sl].reshape(ROWS, 1),
                "table": token_weight,
                "pos": pos_pad[sl],
            }
        )

    res = run_bass_kernel_spmd(nc, in_maps, list(range(NCORES)), trace=TRACE)
    LAST_RESULTS = res
    out = np.concatenate([r["out"] for r in res.results], axis=0)[:SEQ]
    return out[None]



# revision 2
# speedup vs baseline: 1.1212x; 1.1212x over previous
"""CLIP text embedding lookup on 8 TRN2 NeuronCores.

out[1, 77, 768] = token_weight[input_ids] + position_weight[position_ids]

Strategy: sequence-parallel. 77 positions are padded to 80 and split 10 per
core. Each core indirect-DMA-gathers its 10 token rows from the full
replicated [49408, 768] table, adds the core's 10 position-embedding rows
(pre-sharded on the host - position_ids is a static arange; a general
fallback gathers them on the host if a caller passes permuted position_ids),
and writes its [10, 768] output slice. The host concatenates the 8 slices
and trims to 77 rows.

Program (raw bass, no TileContext - saves ~1.1us of barrier framing):
  Pool SWDGE : idx -> SBUF ; gather(table[idx]) -> SBUF ; accum -> out
  SP HWDGE   : pos -> out (parallel, off critical path)
Sem clears run at program start (re-run safe) instead of behind an exit
barrier. Critical path = 3 dependent Pool DMAs; each dependent link costs
~1184 ns (SWDGE descriptor-gen serialization + DMA completion-sem
propagation).
"""

import numpy as np

NCORES = 8
SEQ = 77
DIM = 768
VOCAB = 49408
MAX_POS = 77
ROWS = 10  # ceil(77 / 8)
PAD_SEQ = NCORES * ROWS  # 80

# test.py can flip TRACE; LAST_RESULTS stashes BassKernelResults for test.py.
TRACE = False
LAST_RESULTS = None

_compiled = None


def _build():
    import concourse.bacc as bacc
    import concourse.bass as bass
    import concourse.mybir as mybir

    nc = bacc.Bacc(
        "TRN2", target_bir_lowering=False, debug=False, num_devices=NCORES
    )
    idx = nc.dram_tensor("idx", [ROWS, 1], mybir.dt.int32, kind="ExternalInput").ap()
    table = nc.dram_tensor(
        "table", [VOCAB, DIM], mybir.dt.float32, kind="ExternalInput"
    ).ap()
    pos = nc.dram_tensor(
        "pos", [ROWS, DIM], mybir.dt.float32, kind="ExternalInput"
    ).ap()
    out = nc.dram_tensor(
        "out", [ROWS, DIM], mybir.dt.float32, kind="ExternalOutput"
    ).ap()

    with (
        nc.semaphore("s_idx") as s_idx,
        nc.semaphore("s_pos") as s_pos,
        nc.semaphore("s_gat") as s_gat,
        nc.semaphore("s_out") as s_out,
        nc.sbuf_tensor("idx_t", [ROWS, 1], mybir.dt.int32) as idx_t,
        nc.sbuf_tensor("tok_t", [ROWS, DIM], mybir.dt.float32) as tok_t,
    ):
        # Clear sems at START: re-run-safe (a prior run's 16s are wiped
        # before any wait of this run consumes them; this run's first sem
        # update lands >1.5us later).
        sem_range = range(s_idx.num, s_out.num + 1)
        nc.gpsimd.dma_reset(sem_range)
        nc.gpsimd.sem_clear(sem_range)
        nc.gpsimd.dma_start(out=idx_t[:], in_=idx[:]).then_inc(s_idx, 16)
        nc.sync.dma_start(out=out[:], in_=pos[:]).then_inc(s_pos, 16)
        gat = nc.gpsimd.indirect_dma_start(
            out=tok_t[:],
            out_offset=None,
            in_=table[:],
            in_offset=bass.IndirectOffsetOnAxis(ap=idx_t[:, :1], axis=0),
        )
        gat._wait_ge(s_idx, 16)
        gat.then_inc(s_gat, 16)
        nc.gpsimd.wait_ge(s_pos, 16)
        acc = nc.gpsimd.dma_start(
            out=out[:], in_=tok_t[:], accum_op=mybir.AluOpType.add
        )
        acc._wait_ge(s_gat, 16)
        acc.then_inc(s_out, 16)
        nc.gpsimd.wait_ge(s_out, 16)
    nc.compile()
    return nc


def kernel(**inputs) -> np.ndarray:
    global _compiled, LAST_RESULTS
    from concourse.bass_utils import run_bass_kernel_spmd

    input_ids = np.asarray(inputs["input_ids"]).astype(np.int32).reshape(-1)
    position_ids = np.asarray(inputs["position_ids"]).astype(np.int64).reshape(-1)
    token_weight = np.ascontiguousarray(
        np.asarray(inputs["token_weight"], dtype=np.float32)
    )
    position_weight = np.ascontiguousarray(
        np.asarray(inputs["position_weight"], dtype=np.float32)
    )

    if _compiled is None:
        _compiled = _build()
    nc = _compiled

    ids_pad = np.zeros(PAD_SEQ, np.int32)
    ids_pad[:SEQ] = input_ids
    # Shard the (replicated) position table by sequence position. For the
    # canonical arange position_ids this is a pure row-shard; any other
    # permutation is resolved host-side the same way.
    pos_rows = position_weight[position_ids]  # [SEQ, DIM]
    pos_pad = np.zeros((PAD_SEQ, DIM), np.float32)
    pos_pad[:SEQ] = pos_rows

    in_maps = []
    for c in range(NCORES):
        sl = slice(c * ROWS, (c + 1) * ROWS)
        in_maps.append(
            {
                "idx": ids_pad[sl].reshape(ROWS, 1),
                "table": token_weight,
                "pos": pos_pad[sl],
            }
        )

    res = run_bass_kernel_spmd(nc, in_maps, list(range(NCORES)), trace=TRACE)
    LAST_RESULTS = res
    out = np.concatenate([r["out"] for r in res.results], axis=0)[:SEQ]
    return out[None]


# revision 6
# speedup vs baseline: 1.1684x; 1.0421x over previous
"""CLIP text embedding lookup on 8 TRN2 NeuronCores.

out[1, 77, 768] = token_weight[input_ids] + position_weight[position_ids]

Strategy: sequence-parallel. 77 positions are padded to 80 and split 10 per
core. Each core indirect-DMA-gathers its 10 token rows from the full
replicated [49408, 768] table, adds the core's 10 position-embedding rows
(pre-sharded on the host - position_ids is a static arange; a general
fallback gathers them on the host if a caller passes permuted position_ids),
and writes its [10, 768] output slice. The host concatenates the 8 slices
and trims to 77 rows.

Program (raw bass, no TileContext - saves ~1.1us of barrier framing):
  Pool SWDGE : idx -> SBUF ; gather(table[idx]) -> SBUF ; accum -> out
  SP HWDGE   : pos -> out (parallel, off critical path)
  ACT        : sem clear at program start (re-run safe, off critical path)
The Bass init-time const-tile Memsets and the init all-engine barrier are
suppressed (this kernel uses neither). Critical path = 3 dependent Pool
DMAs; each dependent link costs ~1184 ns (SWDGE descriptor-gen
serialization + DMA completion-sem propagation).
"""

import numpy as np

NCORES = 8
SEQ = 77
DIM = 768
VOCAB = 49408
MAX_POS = 77
ROWS = 10  # ceil(77 / 8)
PAD_SEQ = NCORES * ROWS  # 80

# test.py can flip TRACE; LAST_RESULTS stashes BassKernelResults for test.py.
TRACE = False
LAST_RESULTS = None

_compiled = None


def _build():
    import concourse.bacc as bacc
    import concourse.bass as bass
    import concourse.mybir as mybir

    # Suppress the init-time all-engine barrier (nothing here needs the
    # engine-start sync it provides).
    orig_barrier = bass.Bass.all_engine_barrier
    bass.Bass.all_engine_barrier = lambda self, **kw: None
    try:
        nc = bacc.Bacc(
            "TRN2", target_bir_lowering=False, debug=False, num_devices=NCORES
        )
    finally:
        bass.Bass.all_engine_barrier = orig_barrier
    idx = nc.dram_tensor("idx", [ROWS, 1], mybir.dt.int32, kind="ExternalInput").ap()
    table = nc.dram_tensor(
        "table", [VOCAB, DIM], mybir.dt.float32, kind="ExternalInput"
    ).ap()
    pos = nc.dram_tensor(
        "pos", [ROWS, DIM], mybir.dt.float32, kind="ExternalInput"
    ).ap()
    out = nc.dram_tensor(
        "out", [ROWS, DIM], mybir.dt.float32, kind="ExternalOutput"
    ).ap()

    with (
        nc.semaphore("s_idx") as s_idx,
        nc.semaphore("s_pos") as s_pos,
        nc.semaphore("s_gat") as s_gat,
        nc.semaphore("s_out") as s_out,
        nc.sbuf_tensor("idx_t", [ROWS, 1], mybir.dt.int32) as idx_t,
        nc.sbuf_tensor("tok_t", [ROWS, DIM], mybir.dt.float32) as tok_t,
    ):
        # Clear sems at START: re-run-safe (a prior run's 16s are wiped
        # before any wait of this run consumes them; this run's first sem
        # update lands >1.5us later). On ACT so Pool dispatches immediately.
        sem_range = range(s_idx.num, s_out.num + 1)
        nc.scalar.drain(semaphore_range=sem_range)
        nc.scalar.sem_clear(sem_range)
        nc.gpsimd.dma_start(out=idx_t[:], in_=idx[:]).then_inc(s_idx, 16)
        nc.sync.dma_start(out=out[:], in_=pos[:]).then_inc(s_pos, 16)
        gat = nc.gpsimd.indirect_dma_start(
            out=tok_t[:],
            out_offset=None,
            in_=table[:],
            in_offset=bass.IndirectOffsetOnAxis(ap=idx_t[:, :1], axis=0),
        )
        gat._wait_ge(s_idx, 16)
        gat.then_inc(s_gat, 16)
        nc.gpsimd.wait_ge(s_pos, 16)
        acc = nc.gpsimd.dma_start(
            out=out[:], in_=tok_t[:], accum_op=mybir.AluOpType.add
        )
        acc._wait_ge(s_gat, 16)
        acc.then_inc(s_out, 16)
        nc.gpsimd.wait_ge(s_out, 16)
    # Drop the unused init-time const-tile Memsets from the Pool stream.
    bb0 = nc.main_func.blocks[0]
    bb0.instructions = [
        i for i in bb0.instructions if type(i).__name__ != "InstMemset"
    ]
    nc.compile()
    return nc


def kernel(**inputs) -> np.ndarray:
    global _compiled, LAST_RESULTS
    from concourse.bass_utils import run_bass_kernel_spmd

    input_ids = np.asarray(inputs["input_ids"]).astype(np.int32).reshape(-1)
    position_ids = np.asarray(inputs["position_ids"]).astype(np.int64).reshape(-1)
    token_weight = np.ascontiguousarray(
        np.asarray(inputs["token_weight"], dtype=np.float32)
    )
    position_weight = np.ascontiguousarray(
        np.asarray(inputs["position_weight"], dtype=np.float32)
    )

    if _compiled is None:
        _compiled = _build()
    nc = _compiled

    ids_pad = np.zeros(PAD_SEQ, np.int32)
    ids_pad[:SEQ] = input_ids
    # Shard the (replicated) position table by sequence position. For the
    # canonical arange position_ids this is a pure row-shard; any other
    # permutation is resolved host-side the same way.
    pos_rows = position_weight[position_ids]  # [SEQ, DIM]
    pos_pad = np.zeros((PAD_SEQ, DIM), np.float32)
    pos_pad[:SEQ] = pos_rows

    in_maps = []
    for c in range(NCORES):
        sl = slice(c * ROWS, (c + 1) * ROWS)
        in_maps.append(
            {
                "idx": ids_pad[sl].reshape(ROWS, 1),
                "table": token_weight,
                "pos": pos_pad[sl],
            }
        )

    res = run_bass_kernel_spmd(nc, in_maps, list(range(NCORES)), trace=TRACE)
    LAST_RESULTS = res
    out = np.concatenate([r["out"] for r in res.results], axis=0)[:SEQ]
    return out[None]
